# revision 33
# baseline (speedup 1.0000x reference)
"""Trainium2 Bass kernel for 2D cubic Hermite interpolation (nn_CubicHermite2d).

Math: with x1 = arange(W), x2 = arange(H) (per the problem spec), the whole
op is linear in `signal`:

    result[b, r, q] = sum_{h,w} M2[h, r] * signal[b, h, w] * M1[w, q]

where M1 [W, Nx] / M2 [H, Ny] are 4-banded cubic-Hermite interpolation
matrices built on the host from xs / ys.  Queries are sorted; each query's
4-row source band lives in a 128-ALIGNED window (H/128 = W/128 = 4 windows
total, so the signal loads exactly once and the v intermediate is minimal).
Bands that cross a window boundary get their partner-window rows from one
tiny PSUM-accumulate matmul (start=False) folded behind the big window
matmul - exact math, minimal extra PE work:

    step 1:  v[k][wp, r]  = sig[:, k*128:+128].T @ M2[:, rs:re]   (sum of
             per-h-window K=128 matmuls via the aligned-group structure)
    step 2:  out[b, rm, q] = v[k][:, rm*128:+128].T @ M1[k*128:+128, qs:qe]

Matmuls run in float16 (1 cyc/col on the PE, half the load bytes; ~2^-12
input rounding, measured 1.2e-3 scale-relative absmax vs the fp32
reference, tolerance 2e-2).  The OUTPUT is also stored as f16 (halves the
dominant HBM store traffic: the PSUM->SBUF copies cast f32->f16 for free)
and the host upcasts to f32 after the gather.

Schedule (all measured on HW):
 - ~14.4us of the exec time is fixed bass runtime start/drain (a trivial
   2-DMA kernel measures that); the kernel's own span is ~37us.
 - loads are DMA-descriptor-bound (1KB lines): batch-0 signal rides the
   sync HWDGE ring with strict priority (batches 1.. behind it), weights
   ride the scalar HWDGE ring in parallel; ~16 dummy warmup matmuls during
   the load phase hold the PE HAM clock-gate at 2.4 GHz so the real stream
   starts warm.
 - steady state saturates PE (~31us matmul), DVE+ACT (~29us each of
   PSUM->SBUF cast copies, split by a greedy cost model) and DMA stores
   concurrently; step1 windows of batch b+1 interleave 1:1 between step2
   blocks of batch b.  Batch 0 primes the pipe with fine-grained copies
   and single-r-block first stores.

Sharding: data-parallel over batch B=32 across 8 cores (4 batches/core).
"""

import os
import sys

import numpy as np

for _p in ("/root/.axon_site", "/root/.axon_site/_ro/trn_rl_repo",
           "/root/.axon_site/_ro/pypackages", "/opt/trn_rl_repo"):
    if os.path.isdir(_p) and _p not in sys.path:
        sys.path.append(_p)

import concourse.bass as bass
import concourse.mybir as mybir
from concourse import bacc
from concourse.bass_utils import run_bass_kernel_spmd
from concourse.tile import TileContext

# Problem shapes (hardcoded per spec)
B, H, W = 32, 512, 512
NX, NY = 1024, 1024
N_CORES = 8
NB = B // N_CORES  # batches per core

P = 128
F32 = mybir.dt.float32
# matmul operand dtype: f16 (1 cyc/row, ~2^-12 input rounding, FWL weight
# loads, half DMA bytes) | f32r (2 cyc/row, ~2^-11 rounding) | f32 (4 cyc/row,
# exact) | bf16 (1 cyc/row, ~2^-9 rounding)
MM_MODE = os.environ.get("CH2D_DT", "f16")
_MM_DTS = {"f16": mybir.dt.float16, "bf16": mybir.dt.bfloat16,
           "f32r": mybir.dt.float32r, "f32": mybir.dt.float32}
# output storage dtype: f16 halves the dominant HBM store traffic (outputs
# are O(1); f16 rounding adds ~5e-4 relative, tolerance is 2e-2); the host
# upcasts back to f32 after the gather
OUT_MODE = os.environ.get("CH2D_OUT", "f16")
OUT_DT = _MM_DTS[OUT_MODE] if OUT_MODE != "f32" else mybir.dt.float32
# COARSE_COPY: one [128,1024] PSUM->SBUF copy per tile (2-bank PSUM tiles,
# fewer engine ops) vs two [128,512] copies (1-bank tiles, finer pipelining)
COARSE_COPY = os.environ.get("CH2D_COARSE", "0") == "1"
# V_COARSE: coarse FD=1024 copies for step1's v tiles only (fewer ops/sems
# where slot pressure is low) while step2 keeps fine FD=512 granularity
V_COARSE = os.environ.get("CH2D_VCOARSE", "1") == "1"
VPS_BUFS = int(os.environ.get("CH2D_VPS", "2"))
OPS_BUFS = int(os.environ.get("CH2D_OPS",
                              "2" if COARSE_COPY else ("4" if V_COARSE else "6")))
LOADS_ON_GPSIMD = os.environ.get("CH2D_GPLOAD", "0") == "1"
WARMUP_MMS = int(os.environ.get("CH2D_WARMUP", "16"))
TILE_ENG = os.environ.get("CH2D_TILE_ENG", "0") == "1"
# issue output stores alternately from the Sync and Scalar HWDGE rings to
# halve head-of-line blocking on DMA issue
STORE_SPLIT = os.environ.get("CH2D_STORE_SPLIT", "0") == "1"
# fine-grained (per-PSUM-bank) v copies for every batch, not just batch 0
FINE_ALL = os.environ.get("CH2D_FINE_ALL", "0") == "1"
N_SWDGE = int(os.environ.get("CH2D_SWDGE", "2"))


def _interp_matrix(x0, u):
    """[n, Q] float64 matrix M with (y @ M) == _interp1d(y, x0, slopes, u) of
    the reference (searchsorted bucket, one-sided/averaged Hermite tangents).
    x0 is the sorted sample grid (the reference uses arange, but any sorted
    grid works here)."""
    x0 = np.asarray(x0, dtype=np.float64)
    n = len(x0)
    q = len(u)
    d = np.diff(x0)  # d[j] = x0[j+1] - x0[j]
    m = np.zeros((n, q), dtype=np.float64)
    idx = np.searchsorted(x0[1:-1], u.astype(np.float64))
    dxq = d[idx]
    t = (u.astype(np.float64) - x0[idx]) / dxq
    t2, t3 = t * t, t * t * t
    h00 = 1.0 - 3.0 * t2 + 2.0 * t3
    h10 = (t - 2.0 * t2 + t3) * dxq   # multiplies m[I]
    h01 = 3.0 * t2 - 2.0 * t3
    h11 = (t3 - t2) * dxq             # multiplies m[I+1]
    for k in range(q):
        i = int(idx[k])
        m[i, k] += h00[k]
        m[i + 1, k] += h01[k]
        c = h10[k]  # m[I]: one-sided at 0, averaged interior
        if i == 0:
            m[1, k] += c / d[0]
            m[0, k] -= c / d[0]
        else:
            m[i + 1, k] += 0.5 * c / d[i]
            m[i, k] += 0.5 * c * (1.0 / d[i - 1] - 1.0 / d[i])
            m[i - 1, k] -= 0.5 * c / d[i - 1]
        c = h11[k]  # m[I+1]
        if i + 1 == n - 1:
            m[n - 1, k] += c / d[n - 2]
            m[n - 2, k] -= c / d[n - 2]
        else:
            m[i + 2, k] += 0.5 * c / d[i + 1]
            m[i + 1, k] += 0.5 * c * (1.0 / d[i] - 1.0 / d[i + 1])
            m[i, k] -= 0.5 * c / d[i]
    return m, idx.astype(np.int64)


def _make_groups(idx, n, bank=512):
    """Query groups over 128-ALIGNED source windows (n/P windows total, so
    the signal loads exactly once and v tiles are minimal).  Queries whose
    4-row band crosses a window boundary are folded into the adjacent
    big matmul of their lower window; the rows from the upper window come
    from one tiny PSUM-accumulate matmul per crossing (start=False), so the
    math is exact and the big-matmul count stays minimal.  No group crosses
    a `bank`-multiple in query index (PSUM bank boundary).
    Returns (amms [(qs, qe, win)], bmms [(qs, qe, win, bcol)], n_bcols)."""
    qn = len(idx)
    lo = np.maximum(idx - 1, 0)
    hi = np.minimum(idx + 2, n - 1)
    wa = lo // P
    cross = hi // P != wa
    groups = []
    bcol = 0
    s = 0
    while s < qn:
        e = s
        while e < qn and wa[e] == wa[s]:
            if e > s and (e % bank) == 0:
                break
            e += 1
        # tiny accumulate matmuls for the boundary queries inside this run;
        # they MUST issue right after the big matmul (before the next
        # start=True marks this PSUM zero-region pending-zero again)
        blist = []
        b = s
        while b < e:
            if not cross[b]:
                b += 1
                continue
            be = b
            while be < e and cross[be]:
                be += 1
            blist.append((b, be, int(wa[b]) + 1, bcol))
            bcol += be - b
            b = be
        groups.append((s, int(e), int(wa[s]), tuple(blist)))
        s = e
    return groups, bcol


def _build_nc(g1, g2, nb1, nb2, mm_dt):
    MM_DT = mm_dt
    nc = bacc.Bacc("TRN2", target_bir_lowering=False,
                   name="cubic_hermite2d", num_devices=N_CORES,
                   num_swdge_queues=N_SWDGE)
    nb1 = max(nb1, 8)
    nb2 = max(nb2, 8)
    sig_d = nc.dram_tensor("signal", [NB, H, W], MM_DT, kind="ExternalInput")
    w2a_d = nc.dram_tensor("w2a", [P, NY], MM_DT, kind="ExternalInput")
    w2b_d = nc.dram_tensor("w2b", [P, nb2], MM_DT, kind="ExternalInput")
    w1a_d = nc.dram_tensor("w1a", [P, NX], MM_DT, kind="ExternalInput")
    w1b_d = nc.dram_tensor("w1b", [P, nb1], MM_DT, kind="ExternalInput")
    out_d = nc.dram_tensor("out", [NB, NY, NX], OUT_DT, kind="ExternalOutput")

    # aligned 128-row source windows; the signal tile holds all of them
    # (a window unused by the groups just never gets referenced)
    x_wins = sorted({g[2] for g in g1} | {b[2] for g in g1 for b in g[3]})
    h_wins = list(range(H // P))
    copy_i = 0
    # per-bank halves so PSUM tiles are single-bank
    half1 = [[g for g in g1 if g[1] <= NX // 2], [g for g in g1 if g[0] >= NX // 2]]
    assert sum(map(len, half1)) == len(g1)

    with (
        TileContext(nc) as tc,
        tc.tile_pool(name="const", bufs=1) as const_pool,
        tc.tile_pool(name="sig", bufs=1) as sig_pool,
        tc.tile_pool(name="vbuf", bufs=2 * len(x_wins)) as v_pool,
        tc.tile_pool(name="obuf", bufs=int(os.environ.get("CH2D_OBUF", "6"))) as o_pool,
        tc.tile_pool(name="vps", bufs=VPS_BUFS, space="PSUM") as vps_pool,
        tc.tile_pool(name="ops", bufs=OPS_BUFS, space="PSUM") as ops_pool,
    ):
        load_eng = nc.gpsimd if LOADS_ON_GPSIMD else nc.sync
        # HAM warmup: the PE would otherwise idle at 1.2 GHz (K=4/8) through
        # the initial loads; ~3.4us of dummy matmuls flips it to 2.4 GHz so
        # the real stream starts warm.
        if WARMUP_MMS:
            warm = const_pool.tile([P, 512], MM_DT, name="warm")
            nc.vector.memset(warm[:], 0)
            wps = ops_pool.tile([P, NX // 2], F32, name="ps")
            for i in range(WARMUP_MMS):
                nc.tensor.matmul(out=wps[:, :512], lhsT=warm[:, :P],
                                 rhs=warm[:, :512], start=True, stop=True)

        # critical-path first loads ride the TWO parallel HWDGE rings (sync
        # and scalar): batch-0 signal on sync, weights on scalar.  SWDGE
        # (gpsimd) pays ~2us first-byte latency, so it only carries the big
        # batch 1..NB-1 signal load which isn't needed until later.
        w2a_s = const_pool.tile([P, NY], MM_DT, name="w2as")
        nc.scalar.dma_start(out=w2a_s[:], in_=w2a_d[:, :])
        w2b_s = const_pool.tile([P, nb2], MM_DT, name="w2bs")
        nc.scalar.dma_start(out=w2b_s[:], in_=w2b_d[:, :])

        eng_time = [0.0, 0.0]  # [DVE, ACT] modeled queue time (ns)

        def copy_out(dst, src, eng=None):
            # split PSUM->SBUF copies between DVE and ACT, greedily balancing
            # modeled time: DVE (120+FD)/0.96 ns, ACT (172+FD)/1.2 ns, plus
            # ~150 ns of semaphore overhead per op on each queue
            nonlocal copy_i
            fd = src.free_size()
            cost = [(120 + fd) / 0.96 + 150, (172 + fd) / 1.2 + 150]
            if eng is not None and TILE_ENG:
                e = eng % 2
            else:
                e = 0 if eng_time[0] + cost[0] <= eng_time[1] + cost[1] else 1
            eng_time[e] += cost[e]
            if e == 0:
                nc.vector.tensor_copy(out=dst, in_=src)
            else:
                nc.scalar.copy(out=dst, in_=src)
            copy_i += 1

        # preload signal into ONE tile [128, n_wins, NB, W] (partition p of
        # window-k section holds row k*P+p): batch 0 first as a single
        # strided DMA on sync (small, unblocks the first matmuls fast),
        # then batches 1..NB-1 in one big strided DMA on SWDGE.
        nw = len(h_wins)
        sig_s = sig_pool.tile([P, nw, NB, W], MM_DT, name="sigt")
        src0 = bass.AP(tensor=sig_d, offset=0,
                       ap=[[W, P], [P * W, nw], [1, W]])
        nc.sync.dma_start(out=sig_s[:, :, 0, :], in_=src0)
        w1a_s = const_pool.tile([P, NX], MM_DT, name="w1as")
        nc.scalar.dma_start(out=w1a_s[:], in_=w1a_d[:, :])
        w1b_s = const_pool.tile([P, nb1], MM_DT, name="w1bs")
        nc.scalar.dma_start(out=w1b_s[:], in_=w1b_d[:, :])
        for b in range(1, NB):
            srcr = bass.AP(tensor=sig_d, offset=b * H * W,
                           ap=[[W, P], [P * W, nw], [1, W]])
            load_eng.dma_start(out=sig_s[:, :, b, :], in_=srcr)

        tile_i = [0]

        def build_step1_window(b, k, v_tiles_all, fine=False):
            # v[k][w, r] for w in window k: one [P, NY] tile.  fine=True
            # (pipeline-priming batches) copies per 512-col PSUM bank so the
            # first copy starts before the second bank's matmuls finish.
            vt = v_pool.tile([P, NY], MM_DT, name="vt")
            tile_i[0] += 1
            nhalf = 2 if (fine or FINE_ALL) else 1
            fd = NY // nhalf
            for h in range(nhalf):
                vps = vps_pool.tile([P, fd], F32, name="ps")
                for (rs, re, ka, blist) in g2:
                    if rs < h * fd or rs >= (h + 1) * fd:
                        continue
                    nc.tensor.matmul(
                        out=vps[:, rs - h * fd:re - h * fd],
                        lhsT=sig_s[:, ka, b, k * P:(k + 1) * P],
                        rhs=w2a_s[:, rs:re],
                        start=True, stop=True,
                    )
                    for (bs, be, kb, bc) in blist:
                        nc.tensor.matmul(
                            out=vps[:, bs - h * fd:be - h * fd],
                            lhsT=sig_s[:, kb, b, k * P:(k + 1) * P],
                            rhs=w2b_s[:, bc:bc + be - bs],
                            start=False, stop=True, skip_group_check=True,
                        )
                copy_out(vt[:, h * fd:(h + 1) * fd], vps[:], eng=tile_i[0])
            v_tiles_all.setdefault(b, {})[k] = vt

        def build_step2_block(b, mi_list, v_tiles):
            # one staging tile + one store covering r-blocks mi_list of batch b
            np_ = len(mi_list)
            ot = o_pool.tile([P, np_ * NX], OUT_DT, name="ot",
                             padded_shape=[P, 2 * NX])
            tile_i[0] += 1
            for sub, mi in enumerate(mi_list):
                for hi_, hgroups in enumerate(half1):
                    if not hgroups:
                        continue
                    base = hi_ * (NX // 2)
                    ops = ops_pool.tile([P, NX // 2], F32, name="ps")
                    for (qs, qe, ka, blist) in hgroups:
                        nc.tensor.matmul(
                            out=ops[:, qs - base:qe - base],
                            lhsT=v_tiles[ka][:, mi * P:(mi + 1) * P],
                            rhs=w1a_s[:, qs:qe],
                            start=True, stop=True,
                        )
                        for (bs, be, kb, bc) in blist:
                            nc.tensor.matmul(
                                out=ops[:, bs - base:be - base],
                                lhsT=v_tiles[kb][:, mi * P:(mi + 1) * P],
                                rhs=w1b_s[:, bc:bc + be - bs],
                                start=False, stop=True, skip_group_check=True,
                            )
                    copy_out(ot[:, sub * NX + base:sub * NX + base + NX // 2],
                             ops[:], eng=tile_i[0])
            dst = bass.AP(tensor=out_d,
                          offset=b * NY * NX + mi_list[0] * P * NX,
                          ap=[[NX, P], [P * NX, np_], [1, NX]])
            st_eng = nc.scalar if (STORE_SPLIT and tile_i[0] % 2) else nc.sync
            st_eng.dma_start(out=dst, in_=ot[:])

        v_all = {}
        # software pipeline: interleave the next batch's step1 windows 1:1
        # between the current batch's step2 blocks, so out-copies and
        # v-copies alternate on DVE/ACT and stores flow continuously.
        # Batch 0 primes the pipe: fine-grained v copies + single-r-block
        # first stores.  The final batch stores per-block so the tail
        # drains a smaller store.
        for k in x_wins:
            build_step1_window(0, k, v_all, fine=True)
        for b in range(NB):
            if b + 1 < NB:
                blocks = ([[0], [1], [2, 3], [4, 5], [6, 7]] if b == 0 else
                          [[0, 1], [2, 3], [4, 5], [6, 7]])
                wq = list(x_wins)
                for blk in blocks:
                    build_step2_block(b, blk, v_all[b])
                    if wq:
                        build_step1_window(b + 1, wq.pop(0), v_all)
                for k in wq:
                    build_step1_window(b + 1, k, v_all)
            else:
                for mi in range(NY // P):
                    build_step2_block(b, [mi], v_all[b])

    nc.compile()
    return nc


def _prepare(signal, x1, x2, xs, ys):
    """Host-side prep: sorted-order permutations, interp matrices, groups."""
    xs = np.asarray(xs, dtype=np.float32)
    ys = np.asarray(ys, dtype=np.float32)
    perm_x = None
    if np.any(np.diff(xs) < 0):
        perm_x = np.argsort(xs, kind="stable")
        xs = xs[perm_x]
    perm_y = None
    if np.any(np.diff(ys) < 0):
        perm_y = np.argsort(ys, kind="stable")
        ys = ys[perm_y]

    m1, i1 = _interp_matrix(np.asarray(x1, dtype=np.float64), xs)
    m2, i2 = _interp_matrix(np.asarray(x2, dtype=np.float64), ys)
    g1, nb1 = _make_groups(i1, W)
    g2, nb2 = _make_groups(i2, H)

    def _pack(m, groups, nq, nb):
        # A: each group's primary-window rows; B (packed columns): the
        # boundary queries' partner-window rows.  A+B covers every source row
        # of each query's band exactly once.
        wa = np.zeros((P, nq), dtype=np.float32)
        wb = np.zeros((P, max(nb, 8)), dtype=np.float32)
        for (qs, qe, ka, blist) in groups:
            wa[:, qs:qe] = m[ka * P:(ka + 1) * P, qs:qe]
            for (bs, be, kb, bc) in blist:
                wb[:, bc:bc + be - bs] = m[kb * P:(kb + 1) * P, bs:be]
        return wa, wb

    w1a, w1b = _pack(m1, g1, NX, nb1)
    w2a, w2b = _pack(m2, g2, NY, nb2)
    return g1, g2, nb1, nb2, w1a, w1b, w2a, w2b, perm_x, perm_y


_NC_CACHE = {}


def _run(inputs, trace=False, trace_kwargs=None):
    signal = np.ascontiguousarray(np.asarray(inputs["signal"], dtype=np.float32))
    g1, g2, nb1, nb2, w1a, w1b, w2a, w2b, perm_x, perm_y = _prepare(
        signal, inputs["x1"], inputs["x2"], inputs["xs"], inputs["ys"])

    mm_dt = _MM_DTS[MM_MODE]
    key = (tuple(g1), tuple(g2), mm_dt)
    nc = _NC_CACHE.get(key)
    if nc is None:
        nc = _build_nc(g1, g2, nb1, nb2, mm_dt)
        _NC_CACHE[key] = nc

    np_dt = mybir.dt.np(mm_dt)
    sig_cast = signal.astype(np_dt) if np_dt != np.float32 else signal
    w1ac, w1bc = w1a.astype(np_dt), w1b.astype(np_dt)
    w2ac, w2bc = w2a.astype(np_dt), w2b.astype(np_dt)
    in_maps = []
    for c in range(N_CORES):
        in_maps.append({
            "signal": np.ascontiguousarray(sig_cast[c * NB:(c + 1) * NB]),
            "w2a": w2ac, "w2b": w2bc,
            "w1a": w1ac, "w1b": w1bc,
        })
    res = run_bass_kernel_spmd(
        nc, in_maps, core_ids=list(range(N_CORES)),
        trace=trace, **(trace_kwargs or {}),
    )
    out = np.concatenate([r["out"] for r in res.results], axis=0)
    if out.dtype != np.float32:
        out = out.astype(np.float32)

    # restore original (unsorted) query order if needed
    if perm_y is not None:
        inv = np.empty_like(perm_y)
        inv[perm_y] = np.arange(len(perm_y))
        out = out[:, inv, :]
    if perm_x is not None:
        inv = np.empty_like(perm_x)
        inv[perm_x] = np.arange(len(perm_x))
        out = out[:, :, inv]
    return out, res


def kernel(signal, x1, x2, xs, ys):
    out, _ = _run({"signal": signal, "x1": x1, "x2": x2, "xs": xs, "ys": ys})
    return out

